# revision 6
# baseline (speedup 1.0000x reference)
"""Neural-CA generator kernel for Trainium2 (8 NeuronCores, data parallel).

Problem: 64 steps of a neural cellular automaton on state [N=8, C=80, 32, 32]:
  pre_living  = maxpool3x3(state[:,0]) > 0.01
  perception  = conv3x3(state, W1) + b1          (80 -> 512)
  h           = relu(conv1x1(perception, W2)+b2) (512 -> 512)
  updates     = conv1x1(h, W3) + b3              (512 -> 80)
  state      += updates                          (stochastic mask has p=1)
  post_living = maxpool3x3(state[:,0]) > 0.01
  state      *= (pre_living & post_living)
Output: state[:, :1]  -> (8, 1, 32, 32)

Sharding: pure data parallel, one sample per core; weights replicated.

Per-core layout: channels on SBUF partitions (0..79 live, 80..127 zero),
pixels on the free dim as a zero-padded 34x34 row-major image (1156 elems).
conv3x3 = 9 shifted accumulating matmuls into PSUM; the living-mask
pipeline broadcasts channel 0 to 80 partitions (gpsimd) and computes the
separable 3x3 maxpool with free-dim-shifted max ops, so the mask multiply
is a single elementwise op. No DMAs or transposes inside the step loop.
"""

import sys

sys.path.insert(0, "/opt/trn_rl_repo")

import numpy as np
import concourse.bass as bass
import concourse.bacc as bacc
import concourse.mybir as mybir
import concourse.tile as tile

F32 = mybir.dt.float32
F32R = mybir.dt.float32r

STEPS = 64
N_CORES = 8
C = 80          # state channels
HID = 512
HP = 34         # padded height/width
NP = HP * HP    # 1156
THRESH = 0.01

# interior pixel (r, c) lives at padded linear 34*(r+1) + (c+1); base = 35
INT_BASE = 35


def _interior_ap(t, nh, extra_off=0, rows=16, parts=None):
    """AP over the interior half `nh` (rows 16*nh..) of a padded [P, 1156] tile."""
    ap0 = t[:].ap[0] if parts is None else [t[:].ap[0][0], parts]
    return bass.AP(
        tensor=t[:].tensor,
        offset=t[:].offset + INT_BASE + 544 * nh + extra_off,
        ap=[ap0, [34, rows], [1, 32]],
    )


def build_nc(steps=STEPS, use_f32r=True):
    nc = bacc.Bacc("TRN2", target_bir_lowering=False, debug=False)

    # float32r = PE full-rate fp32 path; every producer of a matmul input
    # must emit it (walrus birverifier enforces rounded-producer dtypes),
    # so the whole activation dataflow uses MD.
    MD = F32R if use_f32r else F32

    state0_d = nc.dram_tensor("state0", [128, NP], MD, kind="ExternalInput")
    w1_d = nc.dram_tensor("w1", [128, 9, 512], MD, kind="ExternalInput")
    w2_d = nc.dram_tensor("w2", [128, 4, 512], MD, kind="ExternalInput")
    w3_d = nc.dram_tensor("w3", [128, 4, 80], MD, kind="ExternalInput")
    b1_d = nc.dram_tensor("b1s", [128, 4], F32, kind="ExternalInput")
    b2_d = nc.dram_tensor("b2s", [128, 4], F32, kind="ExternalInput")
    b3_d = nc.dram_tensor("b3s", [128, 1], F32, kind="ExternalInput")
    out_d = nc.dram_tensor("out0", [1, NP], MD, kind="ExternalOutput")

    with tile.TileContext(nc) as tc:
        with (
            tc.tile_pool(name="pers", bufs=1) as pers,
            tc.tile_pool(name="acts", bufs=2) as acts,
            tc.tile_pool(name="mask", bufs=2) as mask,
            tc.tile_pool(name="ps", bufs=8, space="PSUM") as psp,
        ):
            state = pers.tile([128, NP], MD, tag="state")
            w1s = pers.tile([128, 9, 512], MD, tag="w1s")
            w2s = pers.tile([128, 4, 512], MD, tag="w2s")
            w3s = pers.tile([128, 4, 80], MD, tag="w3s")
            b1t = pers.tile([128, 4], F32, tag="b1t")
            b2t = pers.tile([128, 4], F32, tag="b2t")
            b3t = pers.tile([128, 1], F32, tag="b3t")

            nc.sync.dma_start(state[:], state0_d.ap())
            nc.sync.dma_start(w1s[:], w1_d.ap())
            nc.sync.dma_start(w2s[:], w2_d.ap())
            nc.sync.dma_start(w3s[:], w3_d.ap())
            nc.sync.dma_start(b1t[:], b1_d.ap())
            nc.sync.dma_start(b2t[:], b2_d.ap())
            nc.sync.dma_start(b3t[:], b3_d.ap())

            # conv3x3 shift offsets, k = 3*ky + kx
            deltas = [34 * (ky - 1) + (kx - 1) for ky in range(3) for kx in range(3)]

            def living_chain(tag_sfx):
                """Broadcast state ch0 and compute V = 3x3 zero-padded maxpool
                (valid at interior positions). Returns the V tile."""
                pbc = mask.tile([C, NP], MD, tag=f"pbc{tag_sfx}")
                nc.gpsimd.partition_broadcast(pbc[:], state[0:1, :], channels=C)
                tA = mask.tile([C, NP], MD, tag=f"tA{tag_sfx}")
                tB = mask.tile([C, NP], MD, tag=f"tB{tag_sfx}")
                tC = mask.tile([C, NP], MD, tag=f"tC{tag_sfx}")
                mx = mybir.AluOpType.max
                # t1[n] = max(x[n], x[n+1])
                nc.vector.tensor_tensor(out=tA[:, 0:1155], in0=pbc[:, 0:1155],
                                        in1=pbc[:, 1:1156], op=mx)
                # H[m] = max(t1[m-1], x[m+1]) for m in [1,1155)
                nc.vector.tensor_tensor(out=tB[:, 1:1155], in0=tA[:, 0:1154],
                                        in1=pbc[:, 2:1156], op=mx)
                # u[m] = max(H[m], H[m+34]) for m in [1,1121)
                nc.vector.tensor_tensor(out=tA[:, 1:1121], in0=tB[:, 1:1121],
                                        in1=tB[:, 35:1155], op=mx)
                # V[m] = max(u[m-34], H[m+34]) for m in [35,1121)
                nc.vector.tensor_tensor(out=tC[:, 35:1121], in0=tA[:, 1:1087],
                                        in1=tB[:, 69:1155], op=mx)
                return tC

            for _step in range(steps):
                _perc = acts.tile([128, 4, 1024], MD, tag="perc")
                _hbuf = acts.tile([128, 4, 1024], MD, tag="hbuf")

                # --- pre-living from state ch0 (before update) ---
                v_pre = living_chain("p")
                pre80 = mask.tile([C, 1024], MD, tag="pre80")
                v_pre_int = bass.AP(
                    tensor=v_pre[:].tensor, offset=v_pre[:].offset + INT_BASE,
                    ap=[v_pre[:].ap[0], [34, 32], [1, 32]])
                nc.vector.tensor_scalar(out=pre80[:], in0=v_pre_int, scalar1=THRESH,
                                        scalar2=None, op0=mybir.AluOpType.is_gt)

                # --- conv1: 3x3, 80 -> 512, into 8 PSUM tiles ---
                pp = [[None, None] for _ in range(4)]
                for mc in range(4):
                    for k in range(9):
                        lhs = w1s[:, k, 128 * mc:128 * (mc + 1)]
                        for nh in range(2):
                            if k == 0:
                                pp[mc][nh] = psp.tile([128, 512], F32, tag="ps", name=f"pp{mc}_{nh}")
                            nc.tensor.matmul(
                                pp[mc][nh][:], lhs,
                                _interior_ap(state, nh, deltas[k]),
                                start=(k == 0), stop=(k == 8))
                    for nh in range(2):
                        # perception = psum + b1 (per-partition bias)
                        nc.scalar.activation(
                            _perc[:, mc, 512 * nh:512 * (nh + 1)],
                            pp[mc][nh][:],
                            mybir.ActivationFunctionType.Identity,
                            bias=b1t[:, mc:mc + 1], scale=1.0)

                # --- conv2: 1x1, 512 -> 512, relu ---
                hh = [[None, None] for _ in range(4)]
                for mc2 in range(4):
                    for kc in range(4):
                        lhs = w2s[:, kc, 128 * mc2:128 * (mc2 + 1)]
                        for nh in range(2):
                            if kc == 0:
                                hh[mc2][nh] = psp.tile([128, 512], F32, tag="ps", name=f"hh{mc2}_{nh}")
                            nc.tensor.matmul(
                                hh[mc2][nh][:], lhs,
                                _perc[:, kc, 512 * nh:512 * (nh + 1)],
                                start=(kc == 0), stop=(kc == 3))
                    for nh in range(2):
                        nc.scalar.activation(
                            _hbuf[:, mc2, 512 * nh:512 * (nh + 1)],
                            hh[mc2][nh][:],
                            mybir.ActivationFunctionType.Relu,
                            bias=b2t[:, mc2:mc2 + 1], scale=1.0)

                # --- conv3: 1x1, 512 -> 80 ---
                uu = [None, None]
                for kc in range(4):
                    lhs = w3s[:, kc, 0:80]
                    for nh in range(2):
                        if kc == 0:
                            uu[nh] = psp.tile([128, 512], F32, tag="ps", name=f"uu{nh}")
                        nc.tensor.matmul(
                            uu[nh][0:80, :], lhs,
                            _hbuf[:, kc, 512 * nh:512 * (nh + 1)],
                            start=(kc == 0), stop=(kc == 3))

                # --- state += updates + b3 (in place) ---
                for nh in range(2):
                    si = _interior_ap(state, nh, parts=C)
                    nc.vector.scalar_tensor_tensor(
                        out=si, in0=uu[nh][0:80, :], scalar=b3t[0:80, :], in1=si,
                        op0=mybir.AluOpType.add, op1=mybir.AluOpType.add)

                # --- post-living + mask multiply ---
                v_post = living_chain("q")
                liv = mask.tile([C, 1024], MD, tag="liv")
                v_post_int = bass.AP(
                    tensor=v_post[:].tensor, offset=v_post[:].offset + INT_BASE,
                    ap=[v_post[:].ap[0], [34, 32], [1, 32]])
                nc.vector.scalar_tensor_tensor(
                    out=liv[:], in0=v_post_int, scalar=THRESH, in1=pre80[:],
                    op0=mybir.AluOpType.is_gt, op1=mybir.AluOpType.mult)

                sfull = bass.AP(
                    tensor=state[:].tensor, offset=state[:].offset + INT_BASE,
                    ap=[[state[:].ap[0][0], C], [34, 32], [1, 32]])
                nc.vector.tensor_tensor(
                    out=sfull, in0=sfull,
                    in1=liv[:].rearrange("p (a b) -> p a b", a=32, b=32),
                    op=mybir.AluOpType.mult)

            nc.sync.dma_start(out_d.ap(), state[0:1, :])

    nc.compile()
    return nc


def prep_in_maps(z, W1, b1, W2, b2, W3, b3):
    """Host-side prep: padded per-sample initial state + PE-layout weights."""
    z = np.asarray(z, np.float32)
    W1 = np.asarray(W1, np.float32)
    W2 = np.asarray(W2, np.float32)
    W3 = np.asarray(W3, np.float32)
    b1 = np.asarray(b1, np.float32)
    b2 = np.asarray(b2, np.float32)
    b3 = np.asarray(b3, np.float32)

    w1 = np.zeros((128, 9, 512), np.float32)
    w1[0:80] = W1.transpose(1, 2, 3, 0).reshape(80, 9, 512)
    w2 = W2[:, :, 0, 0].T.reshape(4, 128, 512).transpose(1, 0, 2).copy()
    w3 = W3[:, :, 0, 0].T.reshape(4, 128, 80).transpose(1, 0, 2).copy()
    b1s = b1.reshape(4, 128).T.copy()
    b2s = b2.reshape(4, 128).T.copy()
    b3s = np.zeros((128, 1), np.float32)
    b3s[0:80, 0] = b3

    in_maps = []
    for n in range(N_CORES):
        st = np.zeros((128, HP, HP), np.float32)
        st[1:80, 17, 17] = z[n, 0:79, 0, 0]
        st[0, 17, 17] = 1.0
        in_maps.append({
            "state0": st.reshape(128, NP),
            "w1": w1, "w2": w2, "w3": w3,
            "b1s": b1s, "b2s": b2s, "b3s": b3s,
        })
    return in_maps


_NC_CACHE = {}


def _get_nc(steps=STEPS, use_f32r=True):
    key = (steps, use_f32r)
    if key not in _NC_CACHE:
        _NC_CACHE[key] = build_nc(steps, use_f32r)
    return _NC_CACHE[key]


def kernel(z, W1, b1, W2, b2, W3, b3):
    from concourse.bass_utils import run_bass_kernel_spmd

    nc = _get_nc()
    in_maps = prep_in_maps(z, W1, b1, W2, b2, W3, b3)
    res = run_bass_kernel_spmd(nc, in_maps, core_ids=list(range(N_CORES)))
    out = np.zeros((N_CORES, 1, 32, 32), np.float32)
    for n in range(N_CORES):
        full = np.asarray(res.results[n]["out0"], np.float32).reshape(HP, HP)
        out[n, 0] = full[1:33, 1:33]
    return out


# revision 8
# speedup vs baseline: 1.1999x; 1.1999x over previous
"""Neural-CA generator kernel for Trainium2 (8 NeuronCores, data parallel).

Problem: 64 steps of a neural cellular automaton on state [N=8, C=80, 32, 32]:
  pre_living  = maxpool3x3(state[:,0]) > 0.01
  perception  = conv3x3(state, W1) + b1          (80 -> 512)
  h           = relu(conv1x1(perception, W2)+b2) (512 -> 512)
  updates     = conv1x1(h, W3) + b3              (512 -> 80)
  state      += updates                          (stochastic mask has p=1)
  post_living = maxpool3x3(state[:,0]) > 0.01
  state      *= (pre_living & post_living)
Output: state[:, :1]  -> (8, 1, 32, 32)

Sharding: pure data parallel, one sample per core; weights replicated.

Per-core layout: channels on SBUF partitions (0..79 live, 80..127 zero),
pixels on the free dim as a zero-padded 34x34 row-major image (1156 elems).
conv3x3 = 9 shifted accumulating matmuls into PSUM; the living-mask
pipeline broadcasts channel 0 to 80 partitions (gpsimd) and computes the
separable 3x3 maxpool with free-dim-shifted max ops, so the mask multiply
is a single elementwise op. No DMAs or transposes inside the step loop.
"""

import sys

sys.path.insert(0, "/opt/trn_rl_repo")

import numpy as np
import concourse.bass as bass
import concourse.bacc as bacc
import concourse.mybir as mybir
import concourse.tile as tile

F32 = mybir.dt.float32
F32R = mybir.dt.float32r

STEPS = 64
N_CORES = 8
C = 80          # state channels
HID = 512
HP = 34         # padded height/width
NP = HP * HP    # 1156
THRESH = 0.01

# interior pixel (r, c) lives at padded linear 34*(r+1) + (c+1); base = 35
INT_BASE = 35


def _interior_ap(t, nh, extra_off=0, rows=16, parts=None):
    """AP over the interior half `nh` (rows 16*nh..) of a padded [P, 1156] tile."""
    ap0 = t[:].ap[0] if parts is None else [t[:].ap[0][0], parts]
    return bass.AP(
        tensor=t[:].tensor,
        offset=t[:].offset + INT_BASE + 544 * nh + extra_off,
        ap=[ap0, [34, rows], [1, 32]],
    )


def build_nc(steps=STEPS, use_f32r=True, ablate=()):
    nc = bacc.Bacc("TRN2", target_bir_lowering=False, debug=False)

    # float32r = PE full-rate fp32 path; every producer of a matmul input
    # must emit it (walrus birverifier enforces rounded-producer dtypes),
    # so the whole activation dataflow uses MD.
    MD = F32R if use_f32r else F32

    state0_d = nc.dram_tensor("state0", [128, NP], MD, kind="ExternalInput")
    w1_d = nc.dram_tensor("w1", [128, 9, 512], MD, kind="ExternalInput")
    w2_d = nc.dram_tensor("w2", [128, 4, 512], MD, kind="ExternalInput")
    w3_d = nc.dram_tensor("w3", [128, 4, 80], MD, kind="ExternalInput")
    b1_d = nc.dram_tensor("b1s", [128, 4], F32, kind="ExternalInput")
    b2_d = nc.dram_tensor("b2s", [128, 4], F32, kind="ExternalInput")
    b3_d = nc.dram_tensor("b3s", [128, 1], F32, kind="ExternalInput")
    out_d = nc.dram_tensor("out0", [1, NP], MD, kind="ExternalOutput")

    with tile.TileContext(nc) as tc:
        with (
            tc.tile_pool(name="pers", bufs=1) as pers,
            tc.tile_pool(name="acts", bufs=2) as acts,
            tc.tile_pool(name="mask", bufs=2) as mask,
            tc.tile_pool(name="ps", bufs=8, space="PSUM") as psp,
        ):
            state = pers.tile([128, NP], MD, tag="state")
            w1s = pers.tile([128, 9, 512], MD, tag="w1s")
            w2s = pers.tile([128, 4, 512], MD, tag="w2s")
            w3s = pers.tile([128, 4, 80], MD, tag="w3s")
            b1t = pers.tile([128, 4], F32, tag="b1t")
            b2t = pers.tile([128, 4], F32, tag="b2t")
            b3t = pers.tile([128, 1], F32, tag="b3t")

            nc.sync.dma_start(state[:], state0_d.ap())
            nc.sync.dma_start(w1s[:], w1_d.ap())
            nc.sync.dma_start(w2s[:], w2_d.ap())
            nc.sync.dma_start(w3s[:], w3_d.ap())
            nc.sync.dma_start(b1t[:], b1_d.ap())
            nc.sync.dma_start(b2t[:], b2_d.ap())
            nc.sync.dma_start(b3t[:], b3_d.ap())

            # conv3x3 shift offsets, k = 3*ky + kx
            deltas = [34 * (ky - 1) + (kx - 1) for ky in range(3) for kx in range(3)]

            def living_chain(tag_sfx):
                """Broadcast state ch0 and compute V = 3x3 zero-padded maxpool
                (valid at interior positions). Returns the V tile."""
                pbc = mask.tile([C, NP], MD, tag=f"pbc{tag_sfx}")
                nc.gpsimd.partition_broadcast(pbc[:], state[0:1, :], channels=C)
                tA = mask.tile([C, NP], MD, tag=f"tA{tag_sfx}")
                tB = mask.tile([C, NP], MD, tag=f"tB{tag_sfx}")
                tC = mask.tile([C, NP], MD, tag=f"tC{tag_sfx}")
                mx = mybir.AluOpType.max
                # t1[n] = max(x[n], x[n+1])
                nc.vector.tensor_tensor(out=tA[:, 0:1155], in0=pbc[:, 0:1155],
                                        in1=pbc[:, 1:1156], op=mx)
                # H[m] = max(t1[m-1], x[m+1]) for m in [1,1155)
                nc.vector.tensor_tensor(out=tB[:, 1:1155], in0=tA[:, 0:1154],
                                        in1=pbc[:, 2:1156], op=mx)
                # u[m] = max(H[m], H[m+34]) for m in [1,1121)
                nc.vector.tensor_tensor(out=tA[:, 1:1121], in0=tB[:, 1:1121],
                                        in1=tB[:, 35:1155], op=mx)
                # V[m] = max(u[m-34], H[m+34]) for m in [35,1121)
                nc.vector.tensor_tensor(out=tC[:, 35:1121], in0=tA[:, 1:1087],
                                        in1=tB[:, 69:1155], op=mx)
                return tC

            for _step in range(steps):
                _perc = acts.tile([128, 4, 1024], MD, tag="perc")
                _hbuf = acts.tile([128, 4, 1024], MD, tag="hbuf")

                do_mask = "mask" not in ablate
                do_convs = "convs" not in ablate

                # --- pre-living from state ch0 (before update) ---
                v_pre = living_chain("p") if do_mask else None
                pre80 = mask.tile([C, 1024], MD, tag="pre80", name="pre80") if do_mask else None
                if do_mask:
                  v_pre_int = bass.AP(
                    tensor=v_pre[:].tensor, offset=v_pre[:].offset + INT_BASE,
                    ap=[v_pre[:].ap[0], [34, 32], [1, 32]])
                  nc.vector.tensor_scalar(out=pre80[:], in0=v_pre_int, scalar1=THRESH,
                                          scalar2=None, op0=mybir.AluOpType.is_gt)

                if not do_convs:
                    continue
                # --- conv1: 3x3, 80 -> 512, into 8 PSUM tiles ---
                pp = [[None, None] for _ in range(4)]
                for mc in range(4):
                    for k in range(9):
                        lhs = w1s[:, k, 128 * mc:128 * (mc + 1)]
                        for nh in range(2):
                            if k == 0:
                                pp[mc][nh] = psp.tile([128, 512], F32, tag="ps", name=f"pp{mc}_{nh}")
                            nc.tensor.matmul(
                                pp[mc][nh][:], lhs,
                                _interior_ap(state, nh, deltas[k]),
                                start=(k == 0), stop=(k == 8))
                    for nh in range(2):
                        # perception = psum + b1 (per-partition bias)
                        nc.scalar.activation(
                            _perc[:, mc, 512 * nh:512 * (nh + 1)],
                            pp[mc][nh][:],
                            mybir.ActivationFunctionType.Identity,
                            bias=b1t[:, mc:mc + 1], scale=1.0)

                # --- conv2: 1x1, 512 -> 512, relu ---
                hh = [[None, None] for _ in range(4)]
                for mc2 in range(4):
                    for kc in range(4):
                        lhs = w2s[:, kc, 128 * mc2:128 * (mc2 + 1)]
                        for nh in range(2):
                            if kc == 0:
                                hh[mc2][nh] = psp.tile([128, 512], F32, tag="ps", name=f"hh{mc2}_{nh}")
                            nc.tensor.matmul(
                                hh[mc2][nh][:], lhs,
                                _perc[:, kc, 512 * nh:512 * (nh + 1)],
                                start=(kc == 0), stop=(kc == 3))
                    for nh in range(2):
                        nc.scalar.activation(
                            _hbuf[:, mc2, 512 * nh:512 * (nh + 1)],
                            hh[mc2][nh][:],
                            mybir.ActivationFunctionType.Relu,
                            bias=b2t[:, mc2:mc2 + 1], scale=1.0)

                # --- conv3: 1x1, 512 -> 80 ---
                uu = [None, None]
                for kc in range(4):
                    lhs = w3s[:, kc, 0:80]
                    for nh in range(2):
                        if kc == 0:
                            uu[nh] = psp.tile([128, 512], F32, tag="ps", name=f"uu{nh}")
                        nc.tensor.matmul(
                            uu[nh][0:80, :], lhs,
                            _hbuf[:, kc, 512 * nh:512 * (nh + 1)],
                            start=(kc == 0), stop=(kc == 3))

                # --- state += updates + b3 (in place) ---
                for nh in range(2):
                    si = _interior_ap(state, nh, parts=C)
                    nc.vector.scalar_tensor_tensor(
                        out=si, in0=uu[nh][0:80, :], scalar=b3t[0:80, :], in1=si,
                        op0=mybir.AluOpType.add, op1=mybir.AluOpType.add)

                if not do_mask:
                    continue
                # --- post-living + mask multiply ---
                v_post = living_chain("q")
                liv = mask.tile([C, 1024], MD, tag="liv")
                v_post_int = bass.AP(
                    tensor=v_post[:].tensor, offset=v_post[:].offset + INT_BASE,
                    ap=[v_post[:].ap[0], [34, 32], [1, 32]])
                nc.vector.scalar_tensor_tensor(
                    out=liv[:], in0=v_post_int, scalar=THRESH, in1=pre80[:],
                    op0=mybir.AluOpType.is_gt, op1=mybir.AluOpType.mult)

                sfull = bass.AP(
                    tensor=state[:].tensor, offset=state[:].offset + INT_BASE,
                    ap=[[state[:].ap[0][0], C], [34, 32], [1, 32]])
                nc.vector.tensor_tensor(
                    out=sfull, in0=sfull,
                    in1=liv[:].rearrange("p (a b) -> p a b", a=32, b=32),
                    op=mybir.AluOpType.mult)

            nc.sync.dma_start(out_d.ap(), state[0:1, :])

    nc.compile()
    return nc


def prep_in_maps(z, W1, b1, W2, b2, W3, b3):
    """Host-side prep: padded per-sample initial state + PE-layout weights."""
    z = np.asarray(z, np.float32)
    W1 = np.asarray(W1, np.float32)
    W2 = np.asarray(W2, np.float32)
    W3 = np.asarray(W3, np.float32)
    b1 = np.asarray(b1, np.float32)
    b2 = np.asarray(b2, np.float32)
    b3 = np.asarray(b3, np.float32)

    w1 = np.zeros((128, 9, 512), np.float32)
    w1[0:80] = W1.transpose(1, 2, 3, 0).reshape(80, 9, 512)
    w2 = W2[:, :, 0, 0].T.reshape(4, 128, 512).transpose(1, 0, 2).copy()
    w3 = W3[:, :, 0, 0].T.reshape(4, 128, 80).transpose(1, 0, 2).copy()
    b1s = b1.reshape(4, 128).T.copy()
    b2s = b2.reshape(4, 128).T.copy()
    b3s = np.zeros((128, 1), np.float32)
    b3s[0:80, 0] = b3

    in_maps = []
    for n in range(N_CORES):
        st = np.zeros((128, HP, HP), np.float32)
        st[1:80, 17, 17] = z[n, 0:79, 0, 0]
        st[0, 17, 17] = 1.0
        in_maps.append({
            "state0": st.reshape(128, NP),
            "w1": w1, "w2": w2, "w3": w3,
            "b1s": b1s, "b2s": b2s, "b3s": b3s,
        })
    return in_maps


_NC_CACHE = {}


def _get_nc(steps=STEPS, use_f32r=True):
    key = (steps, use_f32r)
    if key not in _NC_CACHE:
        _NC_CACHE[key] = build_nc(steps, use_f32r)
    return _NC_CACHE[key]


def kernel(z, W1, b1, W2, b2, W3, b3):
    from concourse.bass_utils import run_bass_kernel_spmd

    nc = _get_nc()
    in_maps = prep_in_maps(z, W1, b1, W2, b2, W3, b3)
    res = run_bass_kernel_spmd(nc, in_maps, core_ids=list(range(N_CORES)))
    out = np.zeros((N_CORES, 1, 32, 32), np.float32)
    for n in range(N_CORES):
        full = np.asarray(res.results[n]["out0"], np.float32).reshape(HP, HP)
        out[n, 0] = full[1:33, 1:33]
    return out


# revision 15
# speedup vs baseline: 3.1299x; 2.6084x over previous
"""Neural-CA generator kernel for Trainium2 (8 NeuronCores, data parallel).

Problem: 64 steps of a neural cellular automaton on state [N=8, C=80, 32, 32]:
  pre_living  = maxpool3x3(state[:,0]) > 0.01
  perception  = conv3x3(state, W1) + b1          (80 -> 512)
  h           = relu(conv1x1(perception, W2)+b2) (512 -> 512)
  updates     = conv1x1(h, W3) + b3              (512 -> 80)
  state      += updates                          (stochastic mask has p=1)
  post_living = maxpool3x3(state[:,0]) > 0.01
  state      *= (pre_living & post_living)
Output: state[:, :1]  -> (8, 1, 32, 32)

Sharding: pure data parallel, one sample per core; weights replicated.

Per-core layout: channels on SBUF partitions (0..79 live, 80..127 zero),
pixels on the free dim as a zero-padded 34x34 row-major image (1156 elems).
conv3x3 = 9 shifted accumulating matmuls into PSUM; the living-mask
pipeline broadcasts channel 0 to 80 partitions (gpsimd) and computes the
separable 3x3 maxpool with free-dim-shifted max ops, so the mask multiply
is a single elementwise op. No DMAs or transposes inside the step loop.
"""

import sys

sys.path.insert(0, "/opt/trn_rl_repo")

import numpy as np
import concourse.bass as bass
import concourse.bacc as bacc
import concourse.mybir as mybir
import concourse.tile as tile

F32 = mybir.dt.float32
F32R = mybir.dt.float32r

STEPS = 64
N_CORES = 8
C = 80          # state channels
HID = 512
HP = 34         # padded height/width
NP = HP * HP    # 1156
THRESH = 0.01

# interior pixel (r, c) lives at padded linear 34*(r+1) + (c+1); base = 35
INT_BASE = 35


def _interior_ap(t, nh, extra_off=0, rows=16, parts=None):
    """AP over the interior half `nh` (rows 16*nh..) of a padded [P, 1156] tile."""
    ap0 = t[:].ap[0] if parts is None else [t[:].ap[0][0], parts]
    return bass.AP(
        tensor=t[:].tensor,
        offset=t[:].offset + INT_BASE + 544 * nh + extra_off,
        ap=[ap0, [34, rows], [1, 32]],
    )


def build_nc(steps=STEPS, use_f32r=True, ablate=()):
    nc = bacc.Bacc("TRN2", target_bir_lowering=False, debug=False)

    # float32r = PE full-rate fp32 path; every producer of a matmul input
    # must emit it (walrus birverifier enforces rounded-producer dtypes),
    # so the whole activation dataflow uses MD.
    MD = F32R if use_f32r else F32

    state0_d = nc.dram_tensor("state0", [128, NP], MD, kind="ExternalInput")
    w1_d = nc.dram_tensor("w1", [128, 9, 512], MD, kind="ExternalInput")
    w2_d = nc.dram_tensor("w2", [128, 4, 512], MD, kind="ExternalInput")
    w3_d = nc.dram_tensor("w3", [128, 4, 80], MD, kind="ExternalInput")
    b1_d = nc.dram_tensor("b1s", [128, 4], F32, kind="ExternalInput")
    b2_d = nc.dram_tensor("b2s", [128, 4], F32, kind="ExternalInput")
    b3_d = nc.dram_tensor("b3s", [128, 1], F32, kind="ExternalInput")
    e0_d = nc.dram_tensor("e0", [128, 80], MD, kind="ExternalInput")
    i80_d = nc.dram_tensor("i80", [128, 80], MD, kind="ExternalInput")
    b0_d = nc.dram_tensor("b0", [128, NP], MD, kind="ExternalInput")
    out_d = nc.dram_tensor("out0", [1, NP], MD, kind="ExternalOutput")

    do_mask = "mask" not in ablate

    with tile.TileContext(nc) as tc:
        with (
            tc.tile_pool(name="pers", bufs=1) as pers,
            tc.tile_pool(name="acts", bufs=2) as acts,
            tc.tile_pool(name="mask", bufs=2) as mask,
            tc.tile_pool(name="ps", bufs=4, space="PSUM") as psp,
            tc.tile_pool(name="psu", bufs=1, space="PSUM") as psu,
            tc.tile_pool(name="psd", bufs=1, space="PSUM") as psd,
        ):
            state = pers.tile([128, NP], MD, tag="state")
            btile = pers.tile([128, NP], MD, tag="btile")
            w1s = pers.tile([128, 9, 512], MD, tag="w1s")
            w2s = pers.tile([128, 4, 512], MD, tag="w2s")
            w3s = pers.tile([128, 4, 80], MD, tag="w3s")
            b1t = pers.tile([128, 4], F32, tag="b1t")
            b2t = pers.tile([128, 4], F32, tag="b2t")
            b3t = pers.tile([128, 1], F32, tag="b3t")
            e0t = pers.tile([128, 80], MD, tag="e0t")
            i80t = pers.tile([128, 80], MD, tag="i80t")

            nc.sync.dma_start(e0t[:], e0_d.ap())
            nc.sync.dma_start(i80t[:], i80_d.ap())
            nc.sync.dma_start(btile[:], b0_d.ap())
            nc.sync.dma_start(state[:], state0_d.ap())
            nc.sync.dma_start(w1s[:], w1_d.ap())
            nc.sync.dma_start(w2s[:], w2_d.ap())
            nc.sync.dma_start(w3s[:], w3_d.ap())
            nc.sync.dma_start(b1t[:], b1_d.ap())
            nc.sync.dma_start(b2t[:], b2_d.ap())
            nc.sync.dma_start(b3t[:], b3_d.ap())

            # conv3x3 shift offsets, k = 3*ky + kx
            deltas = [34 * (ky - 1) + (kx - 1) for ky in range(3) for kx in range(3)]

            def dilate(name):
                """3x3 binary dilation of btile row 0 (b map), broadcast to C
                partitions: 9 shifted accumulating matmuls against the row-0
                selector e0 (channels 1..127 of btile are zero).
                Output counts in [0..9]; living test is count > 0.5."""
                pd = psd.tile([C, 1024], F32, tag="pdil", name=name)
                for k in range(9):
                    for nh in range(2):
                        nc.tensor.matmul(
                            pd[:, 512 * nh:512 * (nh + 1)], e0t[:],
                            _interior_ap(btile, nh, deltas[k]),
                            start=(k == 0), stop=(k == 8))
                return pd

            for _step in range(steps):
                _perc = acts.tile([128, 4, 1024], MD, tag="perc")
                _hbuf = acts.tile([128, 4, 1024], MD, tag="hbuf")

                # --- pre-living: dilate current b map (b = state ch0 > thr) ---
                if do_mask:
                    pd_pre = dilate("pd_pre")
                    pre80 = mask.tile([C, 1024], F32, tag="pre80", name="pre80")
                    nc.vector.tensor_scalar(
                        out=pre80[:], in0=pd_pre[:], scalar1=0.5,
                        scalar2=None, op0=mybir.AluOpType.is_gt)

                # --- conv1: 3x3, 80 -> 512 ---
                pp = [[None, None] for _ in range(4)]
                for mc in range(4):
                    for k in range(9):
                        lhs = w1s[:, k, 128 * mc:128 * (mc + 1)]
                        for nh in range(2):
                            if k == 0:
                                pp[mc][nh] = psp.tile([128, 512], F32, tag="ps", name=f"pp{mc}_{nh}")
                            nc.tensor.matmul(
                                pp[mc][nh][:], lhs,
                                _interior_ap(state, nh, deltas[k]),
                                start=(k == 0), stop=(k == 8))
                    for nh in range(2):
                        # perception = psum + b1 (per-partition bias) on DVE,
                        # keeping ScalarE single-function (Relu) so its
                        # activation table never reloads inside the loop.
                        nc.vector.tensor_scalar(
                            out=_perc[:, mc, 512 * nh:512 * (nh + 1)],
                            in0=pp[mc][nh][:],
                            scalar1=b1t[:, mc:mc + 1], scalar2=None,
                            op0=mybir.AluOpType.add)

                # --- conv2: 1x1, 512 -> 512, relu ---
                hh = [[None, None] for _ in range(4)]
                for mc2 in range(4):
                    for kc in range(4):
                        lhs = w2s[:, kc, 128 * mc2:128 * (mc2 + 1)]
                        for nh in range(2):
                            if kc == 0:
                                hh[mc2][nh] = psp.tile([128, 512], F32, tag="ps", name=f"hh{mc2}_{nh}")
                            nc.tensor.matmul(
                                hh[mc2][nh][:], lhs,
                                _perc[:, kc, 512 * nh:512 * (nh + 1)],
                                start=(kc == 0), stop=(kc == 3))
                    for nh in range(2):
                        nc.scalar.activation(
                            _hbuf[:, mc2, 512 * nh:512 * (nh + 1)],
                            hh[mc2][nh][:],
                            mybir.ActivationFunctionType.Relu,
                            bias=b2t[:, mc2:mc2 + 1], scale=1.0)

                # --- conv3: 1x1, 512 -> 80, PLUS identity-accumulate of state
                #     so uu2 = updates + state (pre-bias) ---
                uu2 = psu.tile([C, 1024], F32, tag="uu2", name="uu2")
                for nh in range(2):
                    nc.tensor.matmul(
                        uu2[:, 512 * nh:512 * (nh + 1)], i80t[:],
                        _interior_ap(state, nh),
                        start=True, stop=False)
                for kc in range(4):
                    lhs = w3s[:, kc, 0:80]
                    for nh in range(2):
                        nc.tensor.matmul(
                            uu2[:, 512 * nh:512 * (nh + 1)], lhs,
                            _hbuf[:, kc, 512 * nh:512 * (nh + 1)],
                            start=False, stop=(kc == 3))

                if do_mask:
                    # --- post b map: b = (uu2_ch0 + b3_0 > thr), into btile row 0 ---
                    for nh in range(2):
                        nc.vector.tensor_scalar(
                            out=_interior_ap(btile, nh, parts=1),
                            in0=uu2[0:1, 512 * nh:512 * (nh + 1)],
                            scalar1=b3t[0:1, :], scalar2=THRESH,
                            op0=mybir.AluOpType.add, op1=mybir.AluOpType.is_gt)

                    # --- post dilation + living ---
                    pd_post = dilate("pd_post")
                    liv = mask.tile([C, 1024], F32, tag="liv")
                    nc.vector.scalar_tensor_tensor(
                        out=liv[:], in0=pd_post[:], scalar=0.5, in1=pre80[:],
                        op0=mybir.AluOpType.is_gt, op1=mybir.AluOpType.mult)

                    # --- next-step b map: b ANDed with living (b *= liv) ---
                    nc.vector.tensor_tensor(
                        out=_interior_ap(btile, 0, rows=32, parts=1),
                        in0=_interior_ap(btile, 0, rows=32, parts=1),
                        in1=liv[0:1, :].rearrange("p (a b) -> p a b", a=32, b=32),
                        op=mybir.AluOpType.mult)

                    # --- fused state update: state = (uu2 + b3) * living ---
                    sfull = bass.AP(
                        tensor=state[:].tensor, offset=state[:].offset + INT_BASE,
                        ap=[[state[:].ap[0][0], C], [34, 32], [1, 32]])
                    nc.vector.scalar_tensor_tensor(
                        out=sfull, in0=uu2[:], scalar=b3t[0:80, :], in1=liv[:],
                        op0=mybir.AluOpType.add, op1=mybir.AluOpType.mult)
                else:
                    # timing-ablation path: unmasked state update
                    for nh in range(2):
                        nc.vector.tensor_scalar(
                            out=_interior_ap(state, nh, parts=C),
                            in0=uu2[:, 512 * nh:512 * (nh + 1)],
                            scalar1=b3t[0:80, :], scalar2=None,
                            op0=mybir.AluOpType.add)

            nc.sync.dma_start(out_d.ap(), state[0:1, :])

    nc.compile()
    return nc


def prep_in_maps(z, W1, b1, W2, b2, W3, b3):
    """Host-side prep: padded per-sample initial state + PE-layout weights."""
    z = np.asarray(z, np.float32)
    W1 = np.asarray(W1, np.float32)
    W2 = np.asarray(W2, np.float32)
    W3 = np.asarray(W3, np.float32)
    b1 = np.asarray(b1, np.float32)
    b2 = np.asarray(b2, np.float32)
    b3 = np.asarray(b3, np.float32)

    w1 = np.zeros((128, 9, 512), np.float32)
    w1[0:80] = W1.transpose(1, 2, 3, 0).reshape(80, 9, 512)
    w2 = W2[:, :, 0, 0].T.reshape(4, 128, 512).transpose(1, 0, 2).copy()
    w3 = W3[:, :, 0, 0].T.reshape(4, 128, 80).transpose(1, 0, 2).copy()
    b1s = b1.reshape(4, 128).T.copy()
    b2s = b2.reshape(4, 128).T.copy()
    b3s = np.zeros((128, 1), np.float32)
    b3s[0:80, 0] = b3
    e0 = np.zeros((128, 80), np.float32)
    e0[0, :] = 1.0
    i80 = np.zeros((128, 80), np.float32)
    i80[0:80, 0:80] = np.eye(80, dtype=np.float32)

    in_maps = []
    for n in range(N_CORES):
        st = np.zeros((128, HP, HP), np.float32)
        st[1:80, 17, 17] = z[n, 0:79, 0, 0]
        st[0, 17, 17] = 1.0
        b0 = np.zeros((128, HP, HP), np.float32)
        b0[0] = (st[0] > THRESH).astype(np.float32)
        in_maps.append({
            "state0": st.reshape(128, NP),
            "w1": w1, "w2": w2, "w3": w3,
            "b1s": b1s, "b2s": b2s, "b3s": b3s, "e0": e0,
            "i80": i80, "b0": b0.reshape(128, NP),
        })
    return in_maps


_NC_CACHE = {}


def _get_nc(steps=STEPS, use_f32r=True):
    key = (steps, use_f32r)
    if key not in _NC_CACHE:
        _NC_CACHE[key] = build_nc(steps, use_f32r)
    return _NC_CACHE[key]


def kernel(z, W1, b1, W2, b2, W3, b3):
    from concourse.bass_utils import run_bass_kernel_spmd

    nc = _get_nc()
    in_maps = prep_in_maps(z, W1, b1, W2, b2, W3, b3)
    res = run_bass_kernel_spmd(nc, in_maps, core_ids=list(range(N_CORES)))
    out = np.zeros((N_CORES, 1, 32, 32), np.float32)
    for n in range(N_CORES):
        full = np.asarray(res.results[n]["out0"], np.float32).reshape(HP, HP)
        out[n, 0] = full[1:33, 1:33]
    return out
